# revision 1
# baseline (speedup 1.0000x reference)
"""KAN projection kernel for 8x Trainium2 NeuronCores — v2.

Math: out = silu(x) @ scale_base + einsum('ndg,dog->no', B(x), coef*scale_sp)
with cubic B-splines (GRID=5, K=3 -> 8 basis functions) on a uniform grid
over [-1,1].

Reformulation (validated numerically in mathcheck.py):
 1. silu is smooth: fit it on the spline basis (gamma, max err ~2e-5) and
    fold into the coefficients -> the silu/base matmul plane disappears.
 2. B-splines form a partition of unity (sum_s B_s = 1): the constant
    direction of coefficient space becomes a per-output bias -> plane 7
    disappears. The bias enters the PSUM accumulation as the first matmul
    of each group (ones-plane x bias-row, residual-corrected bf16 rows).
 Net: 9 matmul planes -> 7 (-22% PE work). Planes+weights bf16 (rel err
 ~2.8e-3 << 2e-2 tolerance), halving W DMA and SBUF footprint.

Schedule per core (data-parallel over the 8192 tokens, 1024/core):
 - 4 token supertiles of 256. Per supertile one matmul pass accumulates the
   FULL 2048 outputs in all 8 PSUM banks, so each basis plane is consumed by
   8 matmuls (1.7us) while producing it costs ~1us -> the PE never starves
   and HAM stays warm.
 - Basis planes are built per d-chunk on DVE/Pool/ACT: one-hot cell masks
   (uint8) + blending polys, combined with copy_predicated layers (no adds).
 - W streams as 1MB batched DMAs on the sync HWDGE ring (11-tile prefetch);
   x tiles ride the ACT ring; outputs drain via gpsimd SWDGE; evacuation is
   a plain ScalarE copy (bias already accumulated in PSUM).
"""

import sys

sys.path.insert(0, '/opt/trn_rl_repo')

import numpy as np

import concourse.bass as bass  # noqa: F401  (bass must import before mybir use)
import concourse.mybir as mybir
from concourse import bacc
from concourse.tile import TileContext
from concourse.bass_utils import run_bass_kernel_spmd

F32 = mybir.dt.float32
BF16 = mybir.dt.bfloat16
U8 = mybir.dt.uint8
NP_BF16 = mybir.dt.np(BF16)
ALU = mybir.AluOpType
ACTF = mybir.ActivationFunctionType

D = 1024            # input dim
O = 2048            # output dim
NTOK = 8192         # flattened tokens
NCORES = 8
TPC = NTOK // NCORES  # tokens per core = 1024
TS = 256            # token supertile
NTS = TPC // TS     # supertiles per core = 4
NDC = D // 128      # d chunks = 8
NPL = 7             # spline planes after constant folding
NKI = NDC * NPL     # 56 contraction steps of K=128
NGRP = NKI // 8     # 7 groups of 8 ki per W DMA
OC = 512            # output chunk (one PSUM bank)
NOP = O // (2 * OC)  # oc-pairs = 2
MAGIC = 8388608.0   # 2^23 float32 round-to-nearest trick

_CACHE = {}
TRACE = False
LAST_EXEC_NS = None


def _basis(nc, tmp, bpp, planes, xtile, dc, scale, bias):
    """Emit basis ops for one d-chunk: fills planes[dc*NPL + s] (bf16) for
    s=0..6 with the un-normalized (6x) B-spline values; 1/6 is folded into
    the weights on the host."""
    TSl = xtile.shape[1]

    def ftile(tag):
        return tmp.tile([128, TSl], F32, tag=tag, name=tag)

    def btile(tag):
        return tmp.tile([128, TSl], BF16, tag=tag, name=tag)

    # scratch tiles s0..s3 (f32) are reused aggressively (hand-checked)
    s0 = ftile("s0")   # t, then u2
    s1 = ftile("s1")   # r
    s2 = ftile("s2")   # u (f32)
    s3 = ftile("s3")   # m, then u3 (f32)
    s4 = ftile("s4")   # b1t scratch
    # engine split per measured costs: DVE bf16 327ns / f32 594; ACT 612;
    # Pool 1-input ~700-800 (avoid 2-input Pool: 1111)
    nc.scalar.activation(s0[:], xtile[:], ACTF.Copy, bias=bias, scale=scale)
    nc.gpsimd.tensor_scalar(s1[:], s0[:], MAGIC, MAGIC, ALU.add, ALU.subtract)
    nc.vector.tensor_tensor(s2[:], s0[:], s1[:], ALU.subtract)   # u0 = t - r
    nc.gpsimd.tensor_scalar(s3[:], s2[:], 0.0, None, ALU.is_lt)  # m
    cell = btile("cell")
    nc.vector.tensor_tensor(cell[:], s1[:], s3[:], ALU.subtract)  # cell 0..4
    nc.vector.tensor_tensor(s2[:], s2[:], s3[:], ALU.add)        # u in [0,1)
    oh = []
    for c in range(5):
        ohc = tmp.tile([128, TSl], U8, tag=f"oh{c}", name=f"oh{c}")
        eng = nc.gpsimd if c < 2 else nc.vector
        eng.tensor_scalar(ohc[:], cell[:], float(c), None, ALU.is_equal)
        oh.append(ohc)
    nc.scalar.activation(s0[:], s2[:], ACTF.Square)              # u2 f32
    nc.vector.tensor_tensor(s3[:], s0[:], s2[:], ALU.mult)       # u3 f32
    en = btile("en")
    nc.scalar.activation(en[:], s2[:], ACTF.Copy, bias=1.0, scale=-1.0)  # 1-u
    en2 = btile("en2")
    nc.vector.tensor_tensor(en2[:], en[:], en[:], ALU.mult)      # (1-u)^2
    # 6*B blending polys: b0=(1-u)^3, b1=3u^3-6u^2+4, b2=-3u^3+3u^2+3u+1, b3=u^3
    b0 = btile("b0")
    nc.vector.tensor_tensor(b0[:], en2[:], en[:], ALU.mult)
    w1 = btile("w1")
    nc.gpsimd.tensor_scalar(w1[:], s0[:], -6.0, 4.0, ALU.mult, ALU.add)  # 4-6u^2
    b1 = btile("b1")
    nc.vector.scalar_tensor_tensor(b1[:], s3[:], 3.0, w1[:], ALU.mult,
                                   ALU.add)                      # 3u^3+4-6u^2
    p2 = btile("p2")
    nc.vector.scalar_tensor_tensor(p2[:], s3[:], -1.0, s0[:], ALU.mult,
                                   ALU.add)                      # u2-u3
    q2 = btile("q2")
    nc.scalar.activation(q2[:], s2[:], ACTF.Copy, bias=1.0, scale=3.0)   # 3u+1
    b2 = btile("b2")
    nc.vector.scalar_tensor_tensor(b2[:], p2[:], 3.0, q2[:], ALU.mult,
                                   ALU.add)
    b3 = btile("b3")
    nc.scalar.activation(b3[:], s3[:], ACTF.Copy)                # u^3 -> bf16
    bs = [b0, b1, b2, b3]
    # plane s = b_{s-cell}(u) on its support, else 0. Built as one DVE mult
    # (handles the zero background) + copy_predicated layers: overwrite dst
    # with b_{s-c} wherever cell==c. 19 DVE ops total, no adds.
    for s in range(NPL):
        dst = bpp.tile([128, TSl], BF16, tag="bp", name=f"bp{dc}_{s}")
        planes[dc * NPL + s] = dst
        cs = list(range(max(0, s - 3), min(4, s) + 1))
        nc.gpsimd.memset(dst[:], 0.0)
        for c in cs:
            nc.vector.copy_predicated(dst[:], oh[c][:], bs[s - c][:])

def _build(scale: float, bias: float, loop_reps: int = 1):
    """Per-core kernel: xt (D, TPC) f32, w (128, NGRP, 8, O) bf16,
    bvec (128, O) f32 -> out (TPC, O) f32.

    t = x*scale + bias maps x into knot-index space [0, GRID).
    loop_reps>1 wraps the body in a HW loop (for delta-reps HW timing)."""
    nc = bacc.Bacc(None, target_bir_lowering=False, debug=False)
    with TileContext(nc) as tc:
        with tc.tile_pool(name="dram", bufs=1, space="DRAM") as dram:
            xt = dram.tile([D, TPC], F32, kind="ExternalInput", tag="xt")
            w = dram.tile([128, NGRP, 8, O], BF16, kind="ExternalInput", tag="w")
            wb = dram.tile([128, O], BF16, kind="ExternalInput", tag="wb")
            out = dram.tile([TPC, O], F32, kind="ExternalOutput", tag="out")
            with (
                tc.tile_pool(name="bp", bufs=104) as bpp,
                tc.tile_pool(name="xp", bufs=10) as xpp,
                tc.tile_pool(name="tmp", bufs=2) as tmp,
                tc.tile_pool(name="wp", bufs=11) as wpp,
                tc.tile_pool(name="bv", bufs=4) as bvp,
                tc.tile_pool(name="ev", bufs=8) as evp,
                tc.tile_pool(name="wu", bufs=1) as wup,
                tc.tile_pool(name="ps", bufs=8, space="PSUM") as psp,
            ):
                # PE warmup: dummy matmuls fill the pipeline-fill window so the
                # PE p-state/HAM clock is warm when real matmuls arrive.
                # wu[:, 0:128] doubles as the all-ones plane for the bias
                # matmul (bias is accumulated into PSUM as the first matmul
                # of each group: ones.T @ (bias/128) row-replicated).
                wu = wup.tile([128, OC], BF16, tag="wu")
                nc.vector.memset(wu[:], 1.0)
                pw = psp.tile([128, OC], F32, tag="ps", name="pswarm")
                for _ in range(20):
                    nc.tensor.matmul(pw[:], wu[:, 0:128], wu[:],
                                     start=True, stop=True)
                wbtiles = []
                for oc_i in range(4):
                    wbt = bvp.tile([128, OC], BF16, tag="bv",
                                   name=f"wb{oc_i}")
                    nc.sync.dma_start(wbt[:],
                                      wb[:, oc_i * OC:(oc_i + 1) * OC])
                    wbtiles.append(wbt)

                def body():
                    for ts_i in range(NTS):
                        tok0 = ts_i * TS
                        planes = [None] * NKI
                        xtiles = []
                        for dc in range(NDC):
                            xtile = xpp.tile([128, TS], F32, tag="x",
                                             name=f"x{dc}")
                            nc.scalar.dma_start(
                                xtile[:],
                                xt[dc * 128:(dc + 1) * 128, tok0:tok0 + TS])
                            xtiles.append(xtile)
                        for dc in range(NDC):
                            _basis(nc, tmp, bpp, planes, xtiles[dc], dc,
                                   scale, bias)
                        # single matmul pass accumulating ALL 4 oc chunks:
                        # 8 psum tiles [128, OC] (one bank each) cover the
                        # whole 256-token x 2048-out supertile output. The
                        # bias enters as the first matmul of each group
                        # (ones-plane x bias-row), planes consume at 1.7us
                        # apiece vs ~1us production -> PE never starves.
                        ps = [psp.tile([128, OC], F32, tag="ps",
                                       name=f"ps{tt}_{oc}")
                              for tt in range(TS // 128) for oc in range(4)]
                        for tt in range(TS // 128):
                            for oc in range(4):
                                nc.tensor.matmul(ps[tt * 4 + oc][:],
                                                 wu[:, 0:128], wbtiles[oc][:],
                                                 start=True, stop=False)
                        for g in range(NGRP):
                            wts = []
                            for oc in range(4):
                                wt = wpp.tile([128, 8, OC], BF16, tag="w",
                                              name=f"w{oc}")
                                nc.sync.dma_start(
                                    wt[:],
                                    w[:, g, :, oc * OC:(oc + 1) * OC])
                                wts.append(wt)
                            for k in range(8):
                                ki = g * 8 + k
                                for tt in range(TS // 128):
                                    for oc in range(4):
                                        nc.tensor.matmul(
                                            ps[tt * 4 + oc][:],
                                            planes[ki][:, tt * 128:
                                                       (tt + 1) * 128],
                                            wts[oc][:, k, :],
                                            start=False,
                                            stop=(ki == NKI - 1))
                        for tt in range(TS // 128):
                            for oc in range(4):
                                ev = evp.tile([128, OC], F32, tag="ev")
                                # drain banks on two engines in parallel
                                if oc % 2 == 0:
                                    nc.scalar.copy(ev[:], ps[tt * 4 + oc][:])
                                else:
                                    nc.vector.tensor_copy(
                                        ev[:], ps[tt * 4 + oc][:])
                                nc.gpsimd.dma_start(
                                    out[tok0 + tt * 128:
                                        tok0 + (tt + 1) * 128,
                                        oc * OC:(oc + 1) * OC],
                                    ev[:])

                if loop_reps > 1:
                    ET = mybir.EngineType
                    with tc.For_i(0, loop_reps, 1,
                                  hint_engines=(ET.PE, ET.DVE, ET.Pool,
                                                ET.Activation, ET.SP)):
                        body()
                elif loop_reps < 0:
                    for _ in range(-loop_reps):
                        body()
                else:
                    body()
    nc.compile()
    return nc, xt.name, w.name, wb.name, out.name


def _b_splines_np(x, grid, k):
    """Cox-de Boor in numpy (float64). x: (N,), grid: (M,) -> (N, G+k)."""
    x = x[:, None]
    g = grid[None, :]
    B = ((x >= g[:, :-1]) & (x < g[:, 1:])).astype(np.float64)
    for p in range(1, k + 1):
        left = (x - g[:, :-(p + 1)]) / (g[:, p:-1] - g[:, :-(p + 1)])
        right = (g[:, p + 1:] - x) / (g[:, p + 1:] - g[:, 1:-p])
        B = left * B[:, :-1] + right * B[:, 1:]
    return B


def _pack_host(grid, coef, scale_base, scale_sp):
    """Fold silu + constant direction into the weights; pack for the device.

    Returns (scale, bias, W[128, NGRP, 8, O] bf16, bvec[128, O] f32)."""
    g0 = np.asarray(grid[0], np.float64)          # (G+2K+1,) uniform knots
    h = float(g0[1] - g0[0])
    scale = 1.0 / h
    bias = -float(g0[3]) / h                      # t = (x - knot_K)/h

    # gamma: silu fitted on the 8 B-spline basis functions
    xs = np.linspace(float(g0[3]), float(g0[-4]) - 1e-6, 4001)
    Bs = _b_splines_np(xs, g0, 3)                 # (4001, 8)
    silu = xs / (1.0 + np.exp(-xs))
    gamma = np.linalg.lstsq(Bs, silu, rcond=None)[0]    # (8,)

    gam32 = gamma.astype(np.float32)
    C = (np.asarray(coef, np.float32) * np.asarray(scale_sp, np.float32)[:, :, None]
         + np.asarray(scale_base, np.float32)[:, :, None] * gam32[None, None, :])
    C7 = C[:, :, 7]
    bias_o = C7.sum(axis=0, dtype=np.float64)     # (O,)
    Cp = (C[:, :, :7] - C7[:, :, None]) * np.float32(1.0 / 6.0)

    W = np.empty((128, NGRP, 8, O), NP_BF16)
    for ki in range(NKI):
        dc, pl = divmod(ki, NPL)
        g_, k_ = divmod(ki, 8)
        W[:, g_, k_, :] = Cp[dc * 128:(dc + 1) * 128, :, pl].astype(NP_BF16)
    # ones-plane bias weights: rows sum to bias_o. A plain bf16(bias/128) row
    # replicated 128x quantizes coherently (2% of out std!) — correct the
    # last row with the bf16 residual instead.
    wbias = np.broadcast_to((bias_o / 128.0).astype(NP_BF16), (128, O)).copy()
    wbias[127] = (bias_o
                  - wbias[:127].astype(np.float64).sum(axis=0)).astype(NP_BF16)
    return scale, bias, W, wbias


def kernel(x, grid, coef, scale_base, scale_sp):
    assert x.shape == (4, 2048, D) and x.dtype == np.float32
    scale, bias, W, bvec = _pack_host(grid, coef, scale_base, scale_sp)
    key = (round(scale, 9), round(bias, 9))
    if key not in _CACHE:
        _CACHE[key] = _build(scale, bias)
    nc, xt_name, w_name, bv_name, out_name = _CACHE[key]

    xT = np.ascontiguousarray(x.reshape(NTOK, D).T)  # (D, NTOK)
    in_maps = []
    for c in range(NCORES):
        in_maps.append({
            xt_name: np.ascontiguousarray(xT[:, c * TPC:(c + 1) * TPC]),
            w_name: W,
            bv_name: bvec,
        })
    res = run_bass_kernel_spmd(nc, in_maps, core_ids=list(range(NCORES)),
                               trace=TRACE)
    global LAST_EXEC_NS
    LAST_EXEC_NS = res.exec_time_ns
    out = np.concatenate([res.results[c][out_name] for c in range(NCORES)],
                         axis=0)
    return out.reshape(4, 2048, O)


def _pjrt_exec(nc, in_maps):
    """Build a cached PJRT executable (no donation) + device-resident inputs.
    Returns a zero-arg callable that runs the kernel once on all 8 cores."""
    import jax
    from jax.sharding import Mesh, PartitionSpec
    from jax.experimental.shard_map import shard_map
    import concourse.mybir as _mb
    from concourse.bass2jax import (_bass_exec_p, partition_id_tensor,
                                    install_neuronx_cc_hook)
    install_neuronx_cc_hook()
    partition_name = (nc.partition_id_tensor.name
                      if nc.partition_id_tensor else None)
    in_names, out_names, out_avals, zero_outs = [], [], [], []
    for alloc in nc.m.functions[0].allocations:
        if not isinstance(alloc, _mb.MemoryLocationSet):
            continue
        name = alloc.memorylocations[0].name
        if alloc.kind == "ExternalInput":
            if name != partition_name:
                in_names.append(name)
        elif alloc.kind == "ExternalOutput":
            out_names.append(name)
            shape = tuple(alloc.tensor_shape)
            dtype = _mb.dt.np(alloc.dtype)
            out_avals.append(jax.core.ShapedArray(shape, dtype))
            zero_outs.append(np.zeros(shape, dtype))
    n_params = len(in_names)
    all_names = list(in_names) + out_names
    if partition_name is not None:
        all_names.append(partition_name)

    def _body(*args):
        operands = list(args)
        if partition_name is not None:
            operands.append(partition_id_tensor())
        outs = _bass_exec_p.bind(
            *operands, out_avals=tuple(out_avals), in_names=tuple(all_names),
            out_names=tuple(out_names), lowering_input_output_aliases=(),
            sim_require_finite=True, sim_require_nnan=True, nc=nc)
        return tuple(outs)

    n_cores = len(in_maps)
    devices = jax.devices()[:n_cores]
    mesh = Mesh(np.asarray(devices), ("core",))
    nz = len(zero_outs)
    in_specs = (PartitionSpec("core"),) * (n_params + nz)
    out_specs = (PartitionSpec("core"),) * len(out_names)
    fn = jax.jit(shard_map(_body, mesh=mesh, in_specs=in_specs,
                           out_specs=out_specs, check_rep=False),
                 keep_unused=True)
    concat_in = [np.concatenate([np.asarray(in_maps[c][nm])
                                 for c in range(n_cores)], axis=0)
                 for nm in in_names]
    concat_z = [np.zeros((n_cores * z.shape[0], *z.shape[1:]), z.dtype)
                for z in zero_outs]
    dev_args = [jax.device_put(a) for a in concat_in + concat_z]
    _ = jax.block_until_ready(fn(*dev_args))  # compile+warm

    def run():
        return jax.block_until_ready(fn(*dev_args))
    return run


def hw_time_ns(x, grid, coef, scale_base, scale_sp, r1=1, r2=101, iters=16):
    """Device-resident delta-reps timing.

    Inputs live on device and the PJRT executables are cached. The kernel
    body is repeated r2 times in a hardware loop; per-body time comes from
    PAIRWISE interleaved deltas (r1-call immediately followed by r2-call),
    which cancels the drifting axon dispatch constant (~±50 ms between
    measurement sets). Slightly pessimistic: includes For_i back-edge
    overhead per iteration."""
    import time as _time
    scale, bias, W, wbias = _pack_host(grid, coef, scale_base, scale_sp)
    xT = np.ascontiguousarray(x.reshape(NTOK, D).T)
    runs = {}
    for reps in (r1, r2):
        key = (round(scale, 9), round(bias, 9), reps)
        if key not in _CACHE:
            _CACHE[key] = _build(scale, bias, loop_reps=reps)
        nc = _CACHE[key][0]
        names = _CACHE[key][1:4]
        in_maps = [{names[0]: np.ascontiguousarray(
                        xT[:, c * TPC:(c + 1) * TPC]),
                    names[1]: W, names[2]: wbias} for c in range(NCORES)]
        runs[reps] = _pjrt_exec(nc, in_maps)
    deltas = []
    for _ in range(iters):
        t0 = _time.time()
        runs[r1]()
        t1 = _time.time()
        runs[r2]()
        t2 = _time.time()
        deltas.append(((t2 - t1) - (t1 - t0)) / (r2 - r1))
    deltas.sort()
    med = deltas[len(deltas) // 2]
    print(f"  pairwise deltas us/body: p25 {deltas[len(deltas)//4]*1e6:.0f} "
          f"median {med*1e6:.0f} p75 {deltas[3*len(deltas)//4]*1e6:.0f}")
    return med * 1e9



# revision 15
# speedup vs baseline: 1.5130x; 1.5130x over previous
"""KAN projection kernel for 8x Trainium2 NeuronCores — v3.

Math: out = silu(x) @ scale_base + einsum('ndg,dog->no', B(x), coef*scale_sp)
with cubic B-splines (GRID=5, K=3 -> 8 basis functions) on a uniform grid
over [-1,1].

Reformulation (carried over from v2, validated on HW):
 1. silu is smooth: fit it on the spline basis and fold into the
    coefficients -> the silu/base matmul plane disappears.
 2. B-splines form a partition of unity: the constant direction of
    coefficient space becomes a per-output bias -> plane 7 disappears. The
    bias enters PSUM as the first matmul of each bank (ones-plane x
    bias-row, residual-corrected bf16 rows).
 Net: 9 matmul planes -> 7. Planes+weights bf16 (rel err ~2.8e-3 << 2e-2).

v3 schedule (per core, data-parallel over tokens, 1024/core):
 - 2 token supertiles of 512. Basis planes for BOTH supertiles are built
   once ([128,512] bf16, 112 tiles, ~14MB SBUF) and consumed by 8 matmul
   passes (st, oc): each pass accumulates 512 tokens x 512 outputs over all
   56 contraction planes into 4 PSUM banks. Bank sets alternate between
   passes so PSUM evacuation has a full pass (~30us) of slack.
 - W is streamed once per supertile (2x29.4MB total, vs 4x in v2) as 1MB
   [128,8,512] tiles on the sync HWDGE ring.
 - Basis production avoids GpSimd ALU ops entirely (HW-measured 3.6us per
   tensor_scalar there): cell = floor(t) via one DVE magic-round
   (round(t-0.5)), planes built by a dense one-hot multiply (handles the
   zero background, no memsets) + copy_predicated layers on DVE; polys on
   ACT; the few tensor_tensor mult/sub ops ride Pool only if fast.
 - PE stream: dense LDW+MM pairs (N=512 bf16, ~131ns/MM warm), no idle
   window > ~2us so the HAM clock stays at 8/8.
"""

import sys

sys.path.insert(0, '/opt/trn_rl_repo')

import numpy as np

import concourse.bass as bass  # noqa: F401  (bass must import before mybir use)
import concourse.mybir as mybir
from concourse import bacc
from concourse.tile import TileContext
from concourse.bass_utils import run_bass_kernel_spmd

F32 = mybir.dt.float32
BF16 = mybir.dt.bfloat16
U8 = mybir.dt.uint8
NP_BF16 = mybir.dt.np(BF16)
ALU = mybir.AluOpType
ACTF = mybir.ActivationFunctionType
ET = mybir.EngineType

D = 1024            # input dim
O = 2048            # output dim
NTOK = 8192         # flattened tokens
NCORES = 8
TPC = NTOK // NCORES  # tokens per core = 1024
TS = 512            # token supertile
NST = TPC // TS     # supertiles per core = 2
NDC = D // 128      # d chunks = 8
NPL = 7             # spline planes after constant folding
NKI = NDC * NPL     # 56 contraction steps of K=128
NGRP = NKI // 8     # 7 groups of 8 ki per W DMA tile
OC = 512            # output chunk (one PSUM bank)
NOC = O // OC       # oc chunks = 4
NTT = TS // 128     # token tiles per supertile = 4
MAGIC2 = 12582912.0  # 1.5*2^23: add keeps f32 in the ulp-1 range

_CACHE = {}
TRACE = False
LAST_EXEC_NS = None
LAST_RES = None


def _basis(nc, pools, bpp, planes, xtile, st, dc, scale, bias):
    """Emit basis ops for one (supertile, d-chunk): fills
    planes[dc*NPL + s] (bf16, [128, TS]) for s=0..6 with the un-normalized
    (6x) B-spline values; 1/6 is folded into the weights on the host.

    Engine split (HW-measured costs): ACT for all 1-input affine/square
    ops, DVE for the rest. GpSimd gets only tensor_tensor mult/sub
    ("Add"/"Multiply" class), never tensor_scalar (3.6us/op on HW).
    """
    TSl = xtile.shape[1]
    tmA, tmB, tmC = pools

    def t1(tag, dt=F32):
        return tmA.tile([128, TSl], dt, tag=tag, name=tag)

    def tB(tag, dt=F32):
        return tmB.tile([128, TSl], dt, tag=tag, name=tag)

    def t2(tag, dt=F32):
        return tmC.tile([128, TSl], dt, tag=tag, name=tag)

    # t = x*scale + bias in [0, GRID); tp = t - 0.5 feeds the floor round.
    t = t1("t")
    nc.scalar.activation(t[:], xtile[:], ACTF.Copy, bias=bias, scale=scale)
    tp = t1("tp")
    nc.scalar.activation(tp[:], xtile[:], ACTF.Copy, bias=bias - 0.5,
                         scale=scale)
    # cell = floor(t) a.e. = round(tp) via the 1.5*2^23 magic constant:
    # tp + MAGIC2 stays in [2^23, 2^24) where the f32 ulp is exactly 1, so
    # the add rounds to the integer grid. (2^23 with a -0.5 offset breaks:
    # intermediates below 2^23 round on a half-integer grid.) Rounding t-0.5
    # half-to-even only moves points across a cell boundary where the cubic
    # spline is C^2-continuous -> same value. cell is integral 0..4 ->
    # exact in bf16, keeping the one-hot compares on the DVE 2x path.
    cell = t1("cell", BF16)
    nc.vector.tensor_scalar(cell[:], tp[:], MAGIC2, MAGIC2,
                            ALU.add, ALU.subtract)
    u = tB("u")
    nc.gpsimd.tensor_tensor(u[:], t[:], cell[:], ALU.subtract)  # u in [0,1)
    # one-hot cell masks in bf16 (exact 0/1): the scatter below runs as
    # pure bf16 mult/add at the DVE 2x rate (~405ns/op measured).
    oh = []
    for c in range(5):
        ohc = t2(f"oh{c}", BF16)
        nc.vector.tensor_scalar(ohc[:], cell[:], float(c), None, ALU.is_equal)
        oh.append(ohc)
    u2 = tB("u2")
    nc.scalar.activation(u2[:], u[:], ACTF.Square)
    u3 = tB("u3")
    nc.gpsimd.tensor_tensor(u3[:], u2[:], u[:], ALU.mult)
    en = t1("en")
    nc.scalar.activation(en[:], u[:], ACTF.Copy, bias=1.0, scale=-1.0)
    en2 = t1("en2")
    nc.scalar.activation(en2[:], en[:], ACTF.Square)
    # 6*B blending polys: b0=(1-u)^3, b1=3u^3-6u^2+4, b2=-3u^3+3u^2+3u+1,
    # b3=u^3  (all bf16 -- same precision class as the bf16 planes)
    b0 = t2("b0", BF16)
    nc.gpsimd.tensor_tensor(b0[:], en2[:], en[:], ALU.mult)
    w1 = t1("w1")
    nc.scalar.activation(w1[:], u2[:], ACTF.Copy, bias=4.0, scale=-6.0)
    b1 = t2("b1", BF16)
    nc.vector.scalar_tensor_tensor(b1[:], u3[:], 3.0, w1[:], ALU.mult,
                                   ALU.add)
    p2 = t1("p2")
    nc.gpsimd.tensor_tensor(p2[:], u2[:], u3[:], ALU.subtract)  # u2-u3
    q2 = t1("q2")
    nc.scalar.activation(q2[:], u[:], ACTF.Copy, bias=1.0, scale=3.0)
    b2 = t2("b2", BF16)
    nc.vector.scalar_tensor_tensor(b2[:], p2[:], 3.0, q2[:], ALU.mult,
                                   ALU.add)
    b3 = t2("b3", BF16)
    nc.scalar.activation(b3[:], u3[:], ACTF.Copy)
    bs = [b0, b1, b2, b3]
    # plane s = b_{s-cell}(u) on its support, else 0: sum of disjoint
    # one-hot masked products. All-bf16 mult/add (exact: masks are 0/1 and
    # supports are disjoint, so adds never combine two nonzeros). A few
    # masked products ride Pool to keep DVE under the PE budget.
    for s in range(NPL):
        dst = bpp.tile([128, TSl], BF16, tag="bp", name=f"bp{st}_{dc}_{s}")
        planes[dc * NPL + s] = dst
        cs = list(range(max(0, s - 3), min(4, s) + 1))
        c0 = cs[0]
        nc.vector.tensor_tensor(dst[:], oh[c0][:], bs[s - c0][:], ALU.mult)
        for i, c in enumerate(cs[1:]):
            on_pool = (s, c) in ((3, 2), (4, 2), (4, 3), (5, 3), (6, 4))
            if on_pool:
                mt = t2(f"pmt{i % 2}", BF16)
                nc.gpsimd.tensor_tensor(mt[:], oh[c][:], bs[s - c][:],
                                        ALU.mult)
            else:
                mt = t2(f"mt{i % 2}", BF16)
                nc.vector.tensor_tensor(mt[:], oh[c][:], bs[s - c][:],
                                        ALU.mult)
            nc.vector.tensor_tensor(dst[:], dst[:], mt[:], ALU.add)


def _build(scale: float, bias: float, loop_reps: int = 1):
    """Per-core kernel: xt (D, TPC) f32, w (128, NGRP, 8, O) bf16,
    wb (128, O) bf16 -> out (TPC, O) f32.

    loop_reps>1 wraps the body in a HW loop (for delta-reps HW timing)."""
    nc = bacc.Bacc(None, target_bir_lowering=False, debug=False)
    with TileContext(nc) as tc:
        with tc.tile_pool(name="dram", bufs=1, space="DRAM") as dram:
            xt = dram.tile([D, TPC], F32, kind="ExternalInput", tag="xt")
            w = dram.tile([128, NGRP, 8, O], BF16, kind="ExternalInput",
                          tag="w")
            wb = dram.tile([128, O], BF16, kind="ExternalInput", tag="wb")
            out = dram.tile([TPC, O], F32, kind="ExternalOutput", tag="out")
            with (
                tc.tile_pool(name="bp", bufs=112) as bpp,
                tc.tile_pool(name="xp", bufs=3) as xpp,
                tc.tile_pool(name="tmA", bufs=1) as tmA,
                tc.tile_pool(name="tmB", bufs=2) as tmB,
                tc.tile_pool(name="tmC", bufs=1) as tmC,
                tc.tile_pool(name="wp", bufs=2) as wpp,
                tc.tile_pool(name="bv", bufs=4) as bvp,
                tc.tile_pool(name="ev", bufs=6) as evp,
                tc.tile_pool(name="wu", bufs=1) as wup,
                tc.tile_pool(name="ps", bufs=8, space="PSUM") as psp,
            ):
                # PE warmup: dummy matmuls fill the HAM window so the PE
                # clock is at 8/8 when real matmuls arrive. wu[:, 0:128]
                # doubles as the all-ones plane for the bias matmuls.
                wu = wup.tile([128, OC], BF16, tag="wu")
                nc.vector.memset(wu[:], 1.0)
                pw = psp.tile([128, OC], F32, tag="ps", name="pswarm")
                for _ in range(20):
                    nc.tensor.matmul(pw[:], wu[:, 0:128], wu[:],
                                     start=True, stop=True)
                wbtiles = []
                for oc_i in range(NOC):
                    wbt = bvp.tile([128, OC], BF16, tag="bv",
                                   name=f"wb{oc_i}")
                    nc.sync.dma_start(wbt[:],
                                      wb[:, oc_i * OC:(oc_i + 1) * OC])
                    wbtiles.append(wbt)

                def emit_pass(planes_st, st, ocp):
                    """One matmul pass-pair: 512 tokens x 1024 outs (two oc
                    chunks interleaved), all 56 ki into all 8 PSUM banks.
                    Each plane is consumed over 8 consecutive MMs (~1.7us)
                    which matches the production rate, so the PE trails
                    production without long stalls. Returns psum tiles."""
                    ps = [psp.tile([128, OC], F32, tag="ps",
                                   name=f"ps{st}_{ocp}_{p}")
                          for p in range(2 * NTT)]
                    for tt in range(NTT):
                        for oc_r in range(2):
                            nc.tensor.matmul(ps[tt * 2 + oc_r][:],
                                             wu[:, 0:128],
                                             wbtiles[ocp * 2 + oc_r][:],
                                             start=True, stop=False)
                    o0 = ocp * 2 * OC
                    for g in range(NGRP):
                        wt = wpp.tile([128, 8, 2 * OC], BF16, tag="w",
                                      name=f"w{st}_{ocp}_{g}")
                        nc.sync.dma_start(wt[:],
                                          w[:, g, :, o0:o0 + 2 * OC])
                        for k in range(8):
                            ki = g * 8 + k
                            for tt in range(NTT):
                                for oc_r in range(2):
                                    nc.tensor.matmul(
                                        ps[tt * 2 + oc_r][:],
                                        planes_st[ki][:,
                                                      tt * 128:(tt + 1) * 128],
                                        wt[:, k, oc_r * OC:(oc_r + 1) * OC],
                                        start=False,
                                        stop=(ki == NKI - 1))
                    return ps

                def emit_evac(ps, st, ocp):
                    """Evacuate 8 banks on ACT, out-DMA on the ACT ring."""
                    tok0 = st * TS
                    o0 = ocp * 2 * OC
                    for tt in range(NTT):
                        for oc_r in range(2):
                            ev = evp.tile([128, OC], F32, tag="ev")
                            nc.scalar.copy(ev[:], ps[tt * 2 + oc_r][:])
                            nc.scalar.dma_start(
                                out[tok0 + tt * 128:tok0 + (tt + 1) * 128,
                                    o0 + oc_r * OC:o0 + (oc_r + 1) * OC],
                                ev[:])

                def body():
                    planes = [[None] * NKI for _ in range(NST)]

                    def xload(st, dc):
                        xtile = xpp.tile([128, TS], F32, tag="x",
                                         name=f"x{st}_{dc}")
                        nc.scalar.dma_start(
                            xtile[:],
                            xt[dc * 128:(dc + 1) * 128,
                               st * TS:(st + 1) * TS])
                        return xtile

                    # ST0 basis, then ST1 basis; matmul pass-pairs follow
                    # (PE/SP streams only). Evacs are emitted right after
                    # their pass-pair -- the ACT engine reaches the first
                    # one just as pair 0 finishes trailing ST0 production.
                    for dc in range(NDC):
                        _basis(nc, (tmA, tmB, tmC), bpp, planes[0],
                               xload(0, dc), 0, dc, scale, bias)
                    for dc in range(NDC):
                        _basis(nc, (tmA, tmB, tmC), bpp, planes[1],
                               xload(1, dc), 1, dc, scale, bias)
                    for st in range(NST):
                        for ocp in range(NOC // 2):
                            ps = emit_pass(planes[st], st, ocp)
                            emit_evac(ps, st, ocp)

                if loop_reps > 1:
                    with tc.For_i(0, loop_reps, 1,
                                  hint_engines=(ET.PE, ET.DVE, ET.Pool,
                                                ET.Activation, ET.SP)):
                        body()
                elif loop_reps < 0:
                    for _ in range(-loop_reps):
                        body()
                else:
                    body()
    nc.compile()
    return nc, xt.name, w.name, wb.name, out.name


def _b_splines_np(x, grid, k):
    """Cox-de Boor in numpy (float64). x: (N,), grid: (M,) -> (N, G+k)."""
    x = x[:, None]
    g = grid[None, :]
    B = ((x >= g[:, :-1]) & (x < g[:, 1:])).astype(np.float64)
    for p in range(1, k + 1):
        left = (x - g[:, :-(p + 1)]) / (g[:, p:-1] - g[:, :-(p + 1)])
        right = (g[:, p + 1:] - x) / (g[:, p + 1:] - g[:, 1:-p])
        B = left * B[:, :-1] + right * B[:, 1:]
    return B


def _pack_host(grid, coef, scale_base, scale_sp):
    """Fold silu + constant direction into the weights; pack for the device.

    Returns (scale, bias, W[128, NGRP, 8, O] bf16, wb[128, O] bf16)."""
    g0 = np.asarray(grid[0], np.float64)          # (G+2K+1,) uniform knots
    h = float(g0[1] - g0[0])
    scale = 1.0 / h
    bias = -float(g0[3]) / h                      # t = (x - knot_K)/h

    # gamma: silu fitted on the 8 B-spline basis functions
    xs = np.linspace(float(g0[3]), float(g0[-4]) - 1e-6, 4001)
    Bs = _b_splines_np(xs, g0, 3)                 # (4001, 8)
    silu = xs / (1.0 + np.exp(-xs))
    gamma = np.linalg.lstsq(Bs, silu, rcond=None)[0]    # (8,)

    gam32 = gamma.astype(np.float32)
    C = (np.asarray(coef, np.float32)
         * np.asarray(scale_sp, np.float32)[:, :, None]
         + np.asarray(scale_base, np.float32)[:, :, None]
         * gam32[None, None, :])
    C7 = C[:, :, 7]
    bias_o = C7.sum(axis=0, dtype=np.float64)     # (O,)
    Cp = (C[:, :, :7] - C7[:, :, None]) * np.float32(1.0 / 6.0)

    W = np.empty((128, NGRP, 8, O), NP_BF16)
    for ki in range(NKI):
        dc, pl = divmod(ki, NPL)
        g_, k_ = divmod(ki, 8)
        W[:, g_, k_, :] = Cp[dc * 128:(dc + 1) * 128, :, pl].astype(NP_BF16)
    # ones-plane bias weights: rows sum to bias_o. A plain bf16(bias/128)
    # row replicated 128x quantizes coherently -- correct the last row with
    # the bf16 residual instead.
    wbias = np.broadcast_to((bias_o / 128.0).astype(NP_BF16), (128, O)).copy()
    wbias[127] = (bias_o
                  - wbias[:127].astype(np.float64).sum(axis=0)).astype(NP_BF16)
    return scale, bias, W, wbias


def kernel(x, grid, coef, scale_base, scale_sp):
    assert x.shape == (4, 2048, D) and x.dtype == np.float32
    scale, bias, W, bvec = _pack_host(grid, coef, scale_base, scale_sp)
    key = (round(scale, 9), round(bias, 9))
    if key not in _CACHE:
        _CACHE[key] = _build(scale, bias)
    nc, xt_name, w_name, bv_name, out_name = _CACHE[key]

    xT = np.ascontiguousarray(x.reshape(NTOK, D).T)  # (D, NTOK)
    in_maps = []
    for c in range(NCORES):
        in_maps.append({
            xt_name: np.ascontiguousarray(xT[:, c * TPC:(c + 1) * TPC]),
            w_name: W,
            bv_name: bvec,
        })
    res = run_bass_kernel_spmd(nc, in_maps, core_ids=list(range(NCORES)),
                               trace=TRACE)
    global LAST_EXEC_NS, LAST_RES
    LAST_EXEC_NS = res.exec_time_ns
    LAST_RES = res
    out = np.concatenate([res.results[c][out_name] for c in range(NCORES)],
                         axis=0)
    return out.reshape(4, 2048, O)


def _pjrt_exec(nc, in_maps):
    """Build a cached PJRT executable (no donation) + device-resident
    inputs. Returns a zero-arg callable running the kernel on all 8 cores."""
    import jax
    from jax.sharding import Mesh, PartitionSpec
    from jax.experimental.shard_map import shard_map
    import concourse.mybir as _mb
    from concourse.bass2jax import (_bass_exec_p, partition_id_tensor,
                                    install_neuronx_cc_hook)
    install_neuronx_cc_hook()
    partition_name = (nc.partition_id_tensor.name
                      if nc.partition_id_tensor else None)
    in_names, out_names, out_avals, zero_outs = [], [], [], []
    for alloc in nc.m.functions[0].allocations:
        if not isinstance(alloc, _mb.MemoryLocationSet):
            continue
        name = alloc.memorylocations[0].name
        if alloc.kind == "ExternalInput":
            if name != partition_name:
                in_names.append(name)
        elif alloc.kind == "ExternalOutput":
            out_names.append(name)
            shape = tuple(alloc.tensor_shape)
            dtype = _mb.dt.np(alloc.dtype)
            out_avals.append(jax.core.ShapedArray(shape, dtype))
            zero_outs.append(np.zeros(shape, dtype))
    n_params = len(in_names)
    all_names = list(in_names) + out_names
    if partition_name is not None:
        all_names.append(partition_name)

    def _body(*args):
        operands = list(args)
        if partition_name is not None:
            operands.append(partition_id_tensor())
        outs = _bass_exec_p.bind(
            *operands, out_avals=tuple(out_avals), in_names=tuple(all_names),
            out_names=tuple(out_names), lowering_input_output_aliases=(),
            sim_require_finite=True, sim_require_nnan=True, nc=nc)
        return tuple(outs)

    n_cores = len(in_maps)
    devices = jax.devices()[:n_cores]
    mesh = Mesh(np.asarray(devices), ("core",))
    nz = len(zero_outs)
    in_specs = (PartitionSpec("core"),) * (n_params + nz)
    out_specs = (PartitionSpec("core"),) * len(out_names)
    fn = jax.jit(shard_map(_body, mesh=mesh, in_specs=in_specs,
                           out_specs=out_specs, check_rep=False),
                 keep_unused=True)
    concat_in = [np.concatenate([np.asarray(in_maps[c][nm])
                                 for c in range(n_cores)], axis=0)
                 for nm in in_names]
    concat_z = [np.zeros((n_cores * z.shape[0], *z.shape[1:]), z.dtype)
                for z in zero_outs]
    dev_args = [jax.device_put(a) for a in concat_in + concat_z]
    _ = jax.block_until_ready(fn(*dev_args))  # compile+warm

    def run():
        return jax.block_until_ready(fn(*dev_args))
    return run


def hw_time_ns(x, grid, coef, scale_base, scale_sp, r1=1, r2=101, iters=16):
    """Device-resident delta-reps timing.

    The kernel body is repeated r2 times in a hardware loop; per-body time
    comes from PAIRWISE interleaved deltas (r1-call immediately followed by
    r2-call), which cancels the drifting axon dispatch constant. Slightly
    pessimistic: includes For_i back-edge overhead per iteration."""
    import time as _time
    scale, bias, W, wbias = _pack_host(grid, coef, scale_base, scale_sp)
    xT = np.ascontiguousarray(x.reshape(NTOK, D).T)
    runs = {}
    for reps in (r1, r2):
        key = (round(scale, 9), round(bias, 9), reps)
        if key not in _CACHE:
            _CACHE[key] = _build(scale, bias, loop_reps=reps)
        nc = _CACHE[key][0]
        names = _CACHE[key][1:4]
        in_maps = [{names[0]: np.ascontiguousarray(
                        xT[:, c * TPC:(c + 1) * TPC]),
                    names[1]: W, names[2]: wbias} for c in range(NCORES)]
        runs[reps] = _pjrt_exec(nc, in_maps)
    deltas = []
    for _ in range(iters):
        t0 = _time.time()
        runs[r1]()
        t1 = _time.time()
        runs[r2]()
        t2 = _time.time()
        deltas.append(((t2 - t1) - (t1 - t0)) / (r2 - r1))
    deltas.sort()
    med = deltas[len(deltas) // 2]
    print(f"  pairwise deltas us/body: p25 {deltas[len(deltas)//4]*1e6:.0f} "
          f"median {med*1e6:.0f} p75 {deltas[3*len(deltas)//4]*1e6:.0f}")
    return med * 1e9
